# revision 27
# baseline (speedup 1.0000x reference)
"""Trainium2 Bass kernel for the ActionableRGM rotation-scan model.

Math: reference computes
    theta = cumsum(x @ om^T, axis=L)          [B,L,M]
    w     = [w0_0, rot(theta_m) @ (a_m, b_m)] [B,L,D]   (w0 = S^-1 z0)
    out   = S @ w
Since cumsum commutes with the (linear) projection by om, we host-compute
cumx = cumsum(x, axis=1) ([B,L,2], tiny) and device-compute
    theta' = cumx @ (om/2pi)^T                 (in revolutions)
    s, c   = sin/cos(2pi theta')               (range-reduced on device)
    out    = c @ P^T + s @ Q^T + const         (P,Q fold a,b into S columns)
with P[i,m] = S[i,1+2m] a_m + S[i,2+2m] b_m, Q[i,m] = S[i,2+2m] a_m - S[i,1+2m] b_m,
const = S[:,0] w0_0 (added host-side during the gather).

Sharding: pure data-parallel, batches 4i..4i+3 on core i, no collectives.
Device matmuls run in bf16 (hi/lo split for the angle projection so theta'
keeps ~fp27 precision); accumulation is fp32 in PSUM.
"""
import math

import numpy as np
import ml_dtypes

import concourse.bacc as bacc
import concourse.mybir as mybir
from concourse.tile import TileContext
from concourse import bass_utils

B, L, D = 32, 2048, 513
M = (D - 1) // 2          # 256
NCORES = 8
BS = B // NCORES          # 4 batches per core
LG = 512                  # l-group width (per theta tile)
NG = BS * (L // LG)       # 16 l-groups per core
MAGIC = 12582912.0        # 1.5 * 2^23: fp32 round-to-nearest-int trick
TWO_PI = 2.0 * math.pi

F32 = mybir.dt.float32
BF16 = mybir.dt.bfloat16
ALU = mybir.AluOpType
ACTF = mybir.ActivationFunctionType


def _build():
    nc = bacc.Bacc("TRN2", target_bir_lowering=False, debug=False)

    cumx6_d = nc.dram_tensor("cumx6", [6, BS * L], BF16, kind="ExternalInput")
    th_lhsT_d = nc.dram_tensor("th_lhsT", [6, M], BF16, kind="ExternalInput")
    pqt_d = nc.dram_tensor("pqt", [128, 4 * D], BF16, kind="ExternalInput")
    out_d = nc.dram_tensor("out", [BS, L, D], F32, kind="ExternalOutput")

    HALF_PI = math.pi / 2.0
    with TileContext(nc) as tc:
        with (
            tc.tile_pool(name="const", bufs=1) as cpool,
            tc.tile_pool(name="work", bufs=2) as wpool,
            tc.tile_pool(name="cs", bufs=2) as cspool,
            tc.tile_pool(name="osb", bufs=2) as opool,
            tc.tile_pool(name="thp", bufs=1, space="PSUM") as thpool,
            tc.tile_pool(name="pt", bufs=2, space="PSUM") as ptpool,
        ):
            cumx6 = cpool.tile([6, BS * L], BF16)
            nc.sync.dma_start(out=cumx6[:], in_=cumx6_d.ap())
            th_lhsT = cpool.tile([6, M], BF16)
            nc.sync.dma_start(out=th_lhsT[:], in_=th_lhsT_d.ap())
            pqt = cpool.tile([128, 4 * D], BF16)
            nc.sync.dma_start(out=pqt[:], in_=pqt_d.ap())

            for b in range(BS):
                # cumx6 arrives j-major from the host: col = j*128 + u stands
                # for l = 16u + j.  So theta/trig tiles are j-major too and the
                # matmul weights for (mc, j) are the contiguous 128 columns
                # [mc*2048 + j*128 :+128] -> FWL-eligible, and psum partition
                # p = u covers 16 consecutive DRAM rows for the output DMA.
                c01 = cspool.tile([128, 2 * L], BF16, tag="c")
                s01 = cspool.tile([128, 2 * L], BF16, tag="s")
                for h in range(2):
                    # wide theta tile covering two j-blocks (g4 = 2h, 2h+1):
                    # col = mc*1024 + g'*512 + t
                    th = thpool.tile([128, 2048], F32)
                    for gp in range(2):
                        for mc in range(2):
                            nc.tensor.matmul(
                                th[:, mc * 1024 + gp * 512: mc * 1024 + gp * 512 + 512],
                                th_lhsT[:, mc * 128:(mc + 1) * 128],
                                cumx6[:, b * L + (2 * h + gp) * 512:
                                      b * L + (2 * h + gp) * 512 + 512],
                                start=True, stop=True,
                            )
                    k = wpool.tile([128, 2048], F32, tag="k")
                    nc.vector.tensor_scalar(k[:], th[:], MAGIC, MAGIC, ALU.add, ALU.subtract)
                    r = wpool.tile([128, 2048], F32, tag="r")
                    nc.vector.scalar_tensor_tensor(r[:], k[:], -1.0, th[:], ALU.mult, ALU.add)
                    ra = wpool.tile([128, 2048], F32, tag="ra")
                    nc.vector.add_range_wrap(ra[:], r[:], shift=0.25, bound=0.5, period=1.0)
                    # s = sin(2 pi r); c = cos(2 pi r) = sin(2 pi wrap(r + 1/4))
                    # one op per (s, c): 3D out AP, contiguous 1024-col run per mc
                    in_v = lambda t: t[:].rearrange("p (mc c) -> p mc c", mc=2)
                    out_v = lambda t: t[:].rearrange("p (mc c) -> p mc c", mc=2)[
                        :, :, h * 1024:(h + 1) * 1024]
                    nc.scalar.activation(out_v(s01), in_v(r), ACTF.Sin, scale=TWO_PI)
                    nc.scalar.activation(out_v(c01), in_v(ra), ACTF.Sin, scale=TWO_PI)

                o = opool.tile([128, 16 * D + 4], F32)
                for j in range(16):
                    # psum pair: bank0 cols 0:257 = out cols 0:257,
                    #            bank1 cols 512:768 = out cols 257:513
                    t = ptpool.tile([128, 1024], F32)
                    for kc in range(4):
                        # contiguous weights: psum partition p <-> l = 16p + j
                        src, mc = (c01, kc) if kc < 2 else (s01, kc - 2)
                        lhsT = src[:, mc * L + j * 128: mc * L + (j + 1) * 128]
                        nc.tensor.matmul(t[:, 0:257], lhsT,
                                         pqt[:, kc * D: kc * D + 257],
                                         start=(kc == 0), stop=(kc == 3))
                        nc.tensor.matmul(t[:, 512:768], lhsT,
                                         pqt[:, kc * D + 257: (kc + 1) * D],
                                         start=(kc == 0), stop=(kc == 3))
                    # single-op evacuation via overlapped 514-wide AP: the last
                    # element (bank1 col 768, garbage) lands on the next slot's
                    # col 0 and is overwritten by evac j+1 (j=15 -> padding).
                    in3 = t[:].rearrange("p (seg i) -> p seg i", seg=2)[:, :, 0:257]
                    out3 = o[:, j * D: j * D + 514].rearrange("p (seg i) -> p seg i", seg=2)
                    # alternate engines so a psum release never queues behind
                    # a long SIN (ACT) or range-reduction op (DVE)
                    if j % 2 == 0:
                        nc.vector.tensor_copy(out3, in3)
                    else:
                        nc.scalar.copy(out3, in3)
                nc.sync.dma_start(
                    out=out_d.ap()[b].rearrange("(p j) i -> p j i", p=128),
                    in_=o[:, 0:16 * D].rearrange("p (j i) -> p j i", j=16),
                )
    nc.compile()
    return nc


_NC = None


def _get_nc():
    global _NC
    if _NC is None:
        _NC = _build()
    return _NC


def _bf16(x):
    return np.asarray(x, dtype=np.float32).astype(ml_dtypes.bfloat16)


def _prep(x, S, om, z0):
    """Host-side prep: exact cumsum + tiny dense algebra; returns per-core
    input maps plus the additive constant."""
    x = np.asarray(x, dtype=np.float32)
    S = np.asarray(S, dtype=np.float32)
    om = np.asarray(om, dtype=np.float32)
    z0 = np.asarray(z0, dtype=np.float32)

    S_inv = np.linalg.inv(S)
    w0 = S_inv @ z0
    a, bb = w0[1::2], w0[2::2]
    # P/Q: out[:, i] = sum_m c_m P[i,m] + s_m Q[i,m] + const_i
    Scol = S[:, 1:]                      # [D, 2M]
    P = Scol[:, 0::2] * a + Scol[:, 1::2] * bb     # [D, M]
    Q = Scol[:, 1::2] * a - Scol[:, 0::2] * bb
    const = (S[:, 0] * w0[0]).astype(np.float32)   # [D]
    pqt = np.concatenate([P.T, Q.T], axis=0)       # [2M=512, D]
    # device layout: [128 partitions, (kc, i)] with kc = K-chunk index
    pqt_bf = np.ascontiguousarray(
        _bf16(pqt).reshape(4, 128, D).transpose(1, 0, 2).reshape(128, 4 * D)
    )

    omr = (om / TWO_PI).astype(np.float32)         # [M, 2] in revolutions
    oh = _bf16(omr).astype(np.float32)
    ol = _bf16(omr - oh).astype(np.float32)
    th_lhsT = np.stack(
        [oh[:, 0], oh[:, 1], oh[:, 0], oh[:, 1], ol[:, 0], ol[:, 1]], axis=0
    )                                              # [6, M]
    th_lhsT_bf = _bf16(th_lhsT)

    cumx = np.cumsum(x, axis=1)                    # [B, L, 2] fp32
    ch = _bf16(cumx).astype(np.float32)
    cl = _bf16(cumx - ch).astype(np.float32)
    # rhs rows pair with th_lhsT rows: [chx, chy, clx, cly, chx, chy]
    cumx6 = np.stack(
        [ch[..., 0], ch[..., 1], cl[..., 0], cl[..., 1], ch[..., 0], ch[..., 1]],
        axis=1,
    )                                              # [B, 6, L]
    # j-major reorder: device col j*128 + u holds l = 16u + j
    cumx6 = np.ascontiguousarray(
        cumx6.reshape(B, 6, 128, 16).transpose(0, 1, 3, 2).reshape(B, 6, L)
    )
    cumx6_bf = _bf16(cumx6)

    in_maps = []
    for ci in range(NCORES):
        shard = cumx6_bf[ci * BS:(ci + 1) * BS]            # [BS, 6, L]
        shard = np.ascontiguousarray(
            shard.transpose(1, 0, 2).reshape(6, BS * L)    # [6, (b l)]
        )
        in_maps.append({
            "cumx6": shard,
            "th_lhsT": th_lhsT_bf,
            "pqt": pqt_bf,
        })
    return in_maps, const


def kernel(x, S, om, z0):
    nc = _get_nc()
    in_maps, const = _prep(x, S, om, z0)
    res = bass_utils.run_bass_kernel_spmd(nc, in_maps, core_ids=list(range(NCORES)))
    out = np.concatenate([res.results[i]["out"] for i in range(NCORES)], axis=0)
    out += const[None, None, :]
    z_n = np.ascontiguousarray(out[:, -1])
    return out, z_n


# revision 30
# speedup vs baseline: 1.1650x; 1.1650x over previous
"""Trainium2 Bass kernel for the ActionableRGM rotation-scan model.

Math: reference computes
    theta = cumsum(x @ om^T, axis=L)          [B,L,M]
    w     = [w0_0, rot(theta_m) @ (a_m, b_m)] [B,L,D]   (w0 = S^-1 z0)
    out   = S @ w
Since cumsum commutes with the (linear) projection by om, we host-compute
cumx = cumsum(x, axis=1) ([B,L,2], tiny) and device-compute
    theta' = cumx @ (om/2pi)^T                 (in revolutions)
    s, c   = sin/cos(2pi theta')               (range-reduced on device)
    out    = c @ P^T + s @ Q^T + const         (P,Q fold a,b into S columns)
with P[i,m] = S[i,1+2m] a_m + S[i,2+2m] b_m, Q[i,m] = S[i,2+2m] a_m - S[i,1+2m] b_m,
const = S[:,0] w0_0 (added host-side during the gather).

Sharding: pure data-parallel, batches 4i..4i+3 on core i, no collectives.
Device matmuls run in bf16 (hi/lo split for the angle projection so theta'
keeps ~fp27 precision); accumulation is fp32 in PSUM.
"""
import math

import numpy as np
import ml_dtypes

import concourse.bacc as bacc
import concourse.mybir as mybir
from concourse.tile import TileContext
from concourse import bass_utils

B, L, D = 32, 2048, 513
M = (D - 1) // 2          # 256
NCORES = 8
BS = B // NCORES          # 4 batches per core
LG = 512                  # l-group width (per theta tile)
NG = BS * (L // LG)       # 16 l-groups per core
MAGIC = 12582912.0        # 1.5 * 2^23: fp32 round-to-nearest-int trick
TWO_PI = 2.0 * math.pi

F32 = mybir.dt.float32
BF16 = mybir.dt.bfloat16
ALU = mybir.AluOpType
ACTF = mybir.ActivationFunctionType


def _build():
    nc = bacc.Bacc("TRN2", target_bir_lowering=False, debug=False)

    cumx6_d = nc.dram_tensor("cumx6", [6, BS * L], BF16, kind="ExternalInput")
    th_lhsT_d = nc.dram_tensor("th_lhsT", [6, M], BF16, kind="ExternalInput")
    pqt_d = nc.dram_tensor("pqt", [128, 4 * D], BF16, kind="ExternalInput")
    out_d = nc.dram_tensor("out", [BS, L, D], F32, kind="ExternalOutput")

    HALF_PI = math.pi / 2.0
    with TileContext(nc) as tc:
        with (
            tc.tile_pool(name="const", bufs=1) as cpool,
            tc.tile_pool(name="work", bufs=2) as wpool,
            tc.tile_pool(name="cs", bufs=2) as cspool,
            tc.tile_pool(name="osb", bufs=2) as opool,
            tc.tile_pool(name="thp", bufs=1, space="PSUM") as thpool,
            tc.tile_pool(name="pt", bufs=3, space="PSUM") as ptpool,
        ):
            cumx6 = cpool.tile([6, BS * L], BF16)
            nc.sync.dma_start(out=cumx6[:], in_=cumx6_d.ap())
            th_lhsT = cpool.tile([6, M], BF16)
            nc.sync.dma_start(out=th_lhsT[:], in_=th_lhsT_d.ap())
            pqt = cpool.tile([128, 4 * D], BF16)
            nc.sync.dma_start(out=pqt[:], in_=pqt_d.ap())

            for b in range(BS):
                # cumx6 arrives reordered from the host: within batch/mc, col
                # hb*1024 + j*128 + u stands for l = hb*1024 + 8u + j.  Matmul
                # weights for (mc, hb, j) are contiguous 128 columns, psum
                # partition p = u covers 8 consecutive DRAM rows, and each
                # half-batch is one sequential 2.1 MB DMA.
                c01 = cspool.tile([128, 2 * L], BF16, tag="c")
                s01 = cspool.tile([128, 2 * L], BF16, tag="s")
                for hb in range(2):
                    for qp in range(2):
                        # theta tile: quarter-batch q = 2*hb + qp, cols (mc, t)
                        th = thpool.tile([128, 1024], F32)
                        base = b * L + hb * 1024 + qp * 512
                        for mc in range(2):
                            nc.tensor.matmul(
                                th[:, mc * 512:(mc + 1) * 512],
                                th_lhsT[:, mc * 128:(mc + 1) * 128],
                                cumx6[:, base: base + 512],
                                start=True, stop=True,
                            )
                        k = wpool.tile([128, 1024], F32, tag="k")
                        nc.vector.tensor_scalar(k[:], th[:], MAGIC, MAGIC,
                                                ALU.add, ALU.subtract)
                        r = wpool.tile([128, 1024], F32, tag="r")
                        nc.vector.scalar_tensor_tensor(r[:], k[:], -1.0, th[:],
                                                       ALU.mult, ALU.add)
                        ra = wpool.tile([128, 1024], F32, tag="ra")
                        nc.vector.add_range_wrap(ra[:], r[:], shift=0.25,
                                                 bound=0.5, period=1.0)
                        # s = sin(2 pi r); c = sin(2 pi wrap(r + 1/4)) = cos
                        in_v = lambda t: t[:].rearrange("p (mc c) -> p mc c", mc=2)
                        out_v = lambda t: t[:].rearrange(
                            "p (mc c) -> p mc c", mc=2)[
                            :, :, hb * 1024 + qp * 512: hb * 1024 + qp * 512 + 512]
                        nc.scalar.activation(out_v(s01), in_v(r), ACTF.Sin,
                                             scale=TWO_PI)
                        nc.scalar.activation(out_v(c01), in_v(ra), ACTF.Sin,
                                             scale=TWO_PI)

                for hb in range(2):
                    o = opool.tile([128, 8 * D + 4], F32)
                    for j in range(8):
                        # psum pair: bank0 cols 0:257 = out cols 0:257,
                        #            bank1 cols 512:768 = out cols 257:513
                        t = ptpool.tile([128, 1024], F32)
                        for kc in range(4):
                            src, mc = (c01, kc) if kc < 2 else (s01, kc - 2)
                            lhsT = src[:, mc * L + hb * 1024 + j * 128:
                                       mc * L + hb * 1024 + (j + 1) * 128]
                            nc.tensor.matmul(t[:, 0:257], lhsT,
                                             pqt[:, kc * D: kc * D + 257],
                                             start=(kc == 0), stop=(kc == 3))
                            nc.tensor.matmul(t[:, 512:768], lhsT,
                                             pqt[:, kc * D + 257: (kc + 1) * D],
                                             start=(kc == 0), stop=(kc == 3))
                        # single-op evacuation via overlapped 514-wide AP: the
                        # trailing garbage element lands on the next slot's
                        # col 0 (overwritten by evac j+1; j=7 -> padding).
                        in3 = t[:].rearrange("p (seg i) -> p seg i", seg=2)[:, :, 0:257]
                        out3 = o[:, j * D: j * D + 514].rearrange(
                            "p (seg i) -> p seg i", seg=2)
                        if j % 2 == 0:
                            nc.vector.tensor_copy(out3, in3)
                        else:
                            nc.scalar.copy(out3, in3)
                    nc.sync.dma_start(
                        out=out_d.ap()[b, hb * 1024:(hb + 1) * 1024, :].rearrange(
                            "(p j) i -> p j i", p=128),
                        in_=o[:, 0:8 * D].rearrange("p (j i) -> p j i", j=8),
                    )
    nc.compile()
    return nc


_NC = None


def _get_nc():
    global _NC
    if _NC is None:
        _NC = _build()
    return _NC


def _bf16(x):
    return np.asarray(x, dtype=np.float32).astype(ml_dtypes.bfloat16)


def _prep(x, S, om, z0):
    """Host-side prep: exact cumsum + tiny dense algebra; returns per-core
    input maps plus the additive constant."""
    x = np.asarray(x, dtype=np.float32)
    S = np.asarray(S, dtype=np.float32)
    om = np.asarray(om, dtype=np.float32)
    z0 = np.asarray(z0, dtype=np.float32)

    S_inv = np.linalg.inv(S)
    w0 = S_inv @ z0
    a, bb = w0[1::2], w0[2::2]
    # P/Q: out[:, i] = sum_m c_m P[i,m] + s_m Q[i,m] + const_i
    Scol = S[:, 1:]                      # [D, 2M]
    P = Scol[:, 0::2] * a + Scol[:, 1::2] * bb     # [D, M]
    Q = Scol[:, 1::2] * a - Scol[:, 0::2] * bb
    const = (S[:, 0] * w0[0]).astype(np.float32)   # [D]
    pqt = np.concatenate([P.T, Q.T], axis=0)       # [2M=512, D]
    # device layout: [128 partitions, (kc, i)] with kc = K-chunk index
    pqt_bf = np.ascontiguousarray(
        _bf16(pqt).reshape(4, 128, D).transpose(1, 0, 2).reshape(128, 4 * D)
    )

    omr = (om / TWO_PI).astype(np.float32)         # [M, 2] in revolutions
    oh = _bf16(omr).astype(np.float32)
    ol = _bf16(omr - oh).astype(np.float32)
    th_lhsT = np.stack(
        [oh[:, 0], oh[:, 1], oh[:, 0], oh[:, 1], ol[:, 0], ol[:, 1]], axis=0
    )                                              # [6, M]
    th_lhsT_bf = _bf16(th_lhsT)

    cumx = np.cumsum(x, axis=1)                    # [B, L, 2] fp32
    ch = _bf16(cumx).astype(np.float32)
    cl = _bf16(cumx - ch).astype(np.float32)
    # rhs rows pair with th_lhsT rows: [chx, chy, clx, cly, chx, chy]
    cumx6 = np.stack(
        [ch[..., 0], ch[..., 1], cl[..., 0], cl[..., 1], ch[..., 0], ch[..., 1]],
        axis=1,
    )                                              # [B, 6, L]
    # reorder: device col hb*1024 + j*128 + u holds l = hb*1024 + 8u + j
    cumx6 = np.ascontiguousarray(
        cumx6.reshape(B, 6, 2, 128, 8).transpose(0, 1, 2, 4, 3).reshape(B, 6, L)
    )
    cumx6_bf = _bf16(cumx6)

    in_maps = []
    for ci in range(NCORES):
        shard = cumx6_bf[ci * BS:(ci + 1) * BS]            # [BS, 6, L]
        shard = np.ascontiguousarray(
            shard.transpose(1, 0, 2).reshape(6, BS * L)    # [6, (b l)]
        )
        in_maps.append({
            "cumx6": shard,
            "th_lhsT": th_lhsT_bf,
            "pqt": pqt_bf,
        })
    return in_maps, const


def kernel(x, S, om, z0):
    nc = _get_nc()
    in_maps, const = _prep(x, S, om, z0)
    res = bass_utils.run_bass_kernel_spmd(nc, in_maps, core_ids=list(range(NCORES)))
    out = np.concatenate([res.results[i]["out"] for i in range(NCORES)], axis=0)
    out += const[None, None, :]
    z_n = np.ascontiguousarray(out[:, -1])
    return out, z_n
